# revision 25
# baseline (speedup 1.0000x reference)
"""Deformable Conv1d (B=8, C_in=64, C_out=64, K=5, L_in=16384) on 8 trn2 cores.

Strategy (data-parallel over batch, one batch element per NeuronCore):
  out[o,l] = sum_{c,k} W[o,c,k] * ( w0*x[c,i0] + w1*x[c,i0+1] ) + bias[o]
with T = l + k + off[l,k], i0 = floor(T), w0 = 1-frac, w1 = frac, and
out-of-range taps contributing 0 (handled exactly by a zero-padded table).

v4 — the interpolation gather runs on the DMA engines (SWDGE dma_gather with
transpose) instead of the duty-throttled GpSimd Q7 cores (whose ap_gather
costs ~27 ns/index).  The SWDGE descriptor ring holds 1024 descriptors, so
the gather is issued in 256-index calls (~20 per 1024-l group).  Per core:
  1. A DRAM row table xrow[t] = [xpad[t,:64] | xpad[t+1,:64]] (256 B rows).
     dma_gather(transpose=True) with host-precomputed idx = floor(T)+PAD
     yields matmul-ready tiles g : (128=[x[i0,c] | x[i0+1,c]], l)
     (columns jt*640 + k*128 + lw, tile-major).
  2. Per l-tile and k: one matmul, stationary lhsT = g-slice, moving rhs =
     [WA_k | WD_k] writes PSUM blocks [A_k | D_k] (A_k = g0.W_k, D_k =
     (g1-g0).W_k via the [[W,-W],[0,W]] trick).
  3. DVE residual per PAIR of l-tiles: u = ps * w2 (w2 = [1, frac] pairs,
     host-precomputed per-(l-partition) scalars broadcast on the free dim,
     contiguous-output multiply), tree adds over the 10 blocks, + bias.
  4. One 2 MiB fp16 DMA of the (L,64) result per core; host transposes back.
"""

import os
import sys
import types

import numpy as np

import concourse.bass as bass
import concourse.mybir as mybir
import concourse.tile as tile
from concourse import bacc
from concourse import bass_utils


def _ensure_axon_ntff_hook():
    """Shim antenv.axon_hooks (absent in this image) so trace=True works."""
    try:
        import antenv.axon_hooks  # noqa: F401

        return
    except ImportError:
        pass
    try:
        import antenv

        mod = types.ModuleType("antenv.axon_hooks")
        _hook = [None]
        mod.set_axon_ntff_profile_hook = lambda h: _hook.__setitem__(0, h)
        mod.get_axon_ntff_profile_hook = lambda: _hook[0]
        sys.modules["antenv.axon_hooks"] = mod
        antenv.axon_hooks = mod
        try:
            from trn_agent_boot.trn_boot import _ntff_profile_via_ctypes

            so_path = "/opt/axon/libaxon_pjrt.so"
            if os.path.exists(so_path):
                mod.set_axon_ntff_profile_hook(_ntff_profile_via_ctypes(so_path))
        except Exception:
            pass
    except Exception:
        pass


_ensure_axon_ntff_hook()

# problem constants (hardcoded; kernel.py must be self-contained)
B = 8
C = 64
O = 64
K = 5
L_IN = 16384
L_OUT = 16380
PAD = 16  # covers |offset| < 15; offsets ~ N(0,1) so max |off| ~ 5.5
R = L_IN + 2 * PAD  # table rows
LT = 128  # l-tile size (partition dim)
NT = L_IN // LT  # 128 l-tiles per core
SC = 1024  # l's per gather group (8 tiles)
NSC = L_IN // SC  # 16
NIDX = K * SC  # 5120 indices per group
TCOL = K * LT  # 640 gather columns per l-tile (tile-major layout)
TIDX = 512  # indices per dma_gather call (ring cap 1024; smaller calls
# spread desc-gen across Q7 cores and drains across DMA rings)
F32 = mybir.dt.float32
F16 = mybir.dt.float16
I16 = mybir.dt.int16

_cache = {}


def _build_nc():
    nc = bacc.Bacc(
        "TRN2",
        target_bir_lowering=False,
        debug=False,
        enable_asserts=False,
        num_devices=B,
        num_swdge_queues=2,
        dynamic_dma_scratch_size=65536,
    )
    xrow = nc.dram_tensor("xrow", (R, 128), F16, kind="ExternalInput")
    idxg = nc.dram_tensor("idxg", (128, NSC, NIDX // 16), I16, kind="ExternalInput")
    w2g = nc.dram_tensor("w2g", (128, NT, 2 * K), F16, kind="ExternalInput")
    wxk = nc.dram_tensor("wxk", (K, 128, 128), F16, kind="ExternalInput")
    bias2 = nc.dram_tensor("bias2", (128, 2, O), F16, kind="ExternalInput")
    out_d = nc.dram_tensor("out", (L_IN, O), F16, kind="ExternalOutput")

    with tile.TileContext(nc) as tc:
        with (
            tc.tile_pool(name="const", bufs=1) as cpool,
            tc.tile_pool(name="gath", bufs=2) as gpool,
            tc.tile_pool(name="work", bufs=4) as wpool,
            tc.tile_pool(name="outp", bufs=1) as opool,
            tc.tile_pool(name="ps", bufs=2, space="PSUM") as pspool,
        ):
            # ---- load constants ----
            idx_t = cpool.tile([128, NSC, NIDX // 16], I16, tag="idx")
            for sc0 in range(NSC):
                nc.sync.dma_start(idx_t[:, sc0, :], idxg[:, sc0, :])
            wxk_t = cpool.tile([128, K, 128], F16, tag="wxk")
            for kk in range(K):
                nc.sync.dma_start(wxk_t[:, kk, :], wxk[kk])
            bias_t = cpool.tile([128, 2, O], F16, tag="bias")
            nc.sync.dma_start(bias_t[:], bias2[:])
            w2_t = cpool.tile([128, NT, 2 * K], F16, tag="w2")
            nc.sync.dma_start(w2_t[:], w2g[:])

            osb = opool.tile([128, NT, O], F16, tag="osb")

            for sc in range(NSC):
                g = gpool.tile([128, NIDX], F16, tag="g")
                for q in range(NIDX // TIDX):
                    nc.gpsimd.dma_gather(
                        g[:, q * TIDX : (q + 1) * TIDX].rearrange(
                            "p (one n) -> p one n", one=1
                        ),
                        xrow[:],
                        idx_t[:, sc, q * (TIDX // 16) : (q + 1) * (TIDX // 16)],
                        num_idxs=TIDX,
                        num_idxs_reg=TIDX,
                        elem_size=128,
                        transpose=True,
                        queue_num=q % 2,
                    )
                for jj in range(0, SC // LT, 2):
                    j = sc * (SC // LT) + jj
                    ps = pspool.tile([128, 1280], F32, tag="ps")
                    for t in range(2):
                        for k in range(K):
                            c0 = (jj + t) * TCOL + k * 128
                            nc.tensor.matmul(
                                ps[:, t * 640 + 128 * k : t * 640 + 128 * k + 128],
                                g[:, c0 : c0 + 128],
                                wxk_t[:, k, :],
                                start=True,
                                stop=True,
                            )
                    # residual: u = ps * w2 ; tree-sum the 10 blocks ; + bias
                    u = wpool.tile([128, 2, 2 * K, O], F16, tag="u")
                    nc.vector.tensor_tensor(
                        u[:],
                        ps[:].rearrange("p (t r o) -> p t r o", t=2, o=O),
                        w2_t[:, j : j + 2, :].to_broadcast((128, 2, 2 * K, O)),
                        mybir.AluOpType.mult,
                    )
                    v = wpool.tile([128, 2, K, O], F16, tag="v")
                    nc.vector.tensor_add(v[:], u[:, :, 0:K, :], u[:, :, K : 2 * K, :])
                    w = wpool.tile([128, 2, 2, O], F16, tag="w")
                    nc.vector.tensor_add(w[:], v[:, :, 0:2, :], v[:, :, 2:4, :])
                    y = wpool.tile([128, 2, O], F16, tag="y")
                    nc.vector.tensor_add(y[:], w[:, :, 0, :], w[:, :, 1, :])
                    nc.vector.tensor_add(y[:], y[:], v[:, :, 4, :])
                    nc.vector.tensor_add(osb[:, j : j + 2, :], y[:], bias_t[:])
                # stream this group's output while later groups compute
                nc.sync.dma_start(
                    out_d[:].rearrange("(j p) o -> p j o", p=128)[
                        :, sc * 8 : (sc + 1) * 8, :
                    ],
                    osb[:, sc * 8 : (sc + 1) * 8, :],
                )
    nc.compile()
    return nc


def _host_prep(x, offsets, weight, bias):
    x = np.asarray(x, np.float32)
    offsets = np.asarray(offsets, np.float32)
    weight = np.asarray(weight, np.float32)
    bias = np.asarray(bias, np.float32)

    # weights: [A_k | D_k] layout; rows 0:64 tap0 -> [W | -W], rows 64:128
    # tap1 -> [0 | W]
    w16 = weight.astype(np.float16)  # (O, C, K)
    wxk = np.zeros((K, 128, 128), np.float16)
    for k in range(K):
        wxk[k, 0:64, 0:64] = w16[:, :, k].T
        wxk[k, 0:64, 64:128] = -w16[:, :, k].T
        wxk[k, 64:128, 64:128] = w16[:, :, k].T
    bias2 = np.broadcast_to(bias.astype(np.float16), (128, 2, O)).copy()

    l_all = np.arange(L_IN, dtype=np.float64)[:, None]  # (L, 1)
    k_all = np.arange(K, dtype=np.float64)[None, :]  # (1, K)

    in_maps = []
    for b in range(B):
        xt = x[b].T  # (L_IN, C)
        xpad = np.zeros((R + 1, C), np.float32)
        xpad[PAD : PAD + L_IN] = xt
        xp16 = xpad.astype(np.float16)
        xrow = np.zeros((R, 128), np.float16)
        xrow[:, 0:64] = xp16[0:R]
        xrow[:, 64:128] = xp16[1 : R + 1]

        off_b = offsets[b, 0]  # (L_OUT, K) f32
        off_pad = np.zeros((L_IN, K), np.float32)
        off_pad[:L_OUT] = off_b
        T = (l_all + k_all + PAD) + off_pad.astype(np.float64)  # (L, K)
        i0f = np.floor(T)
        fr = (T - i0f).astype(np.float32)  # consistent with i0 by construction
        i0 = np.clip(i0f, 0.0, float(R - 2)).astype(np.int16)  # (L, K)

        # gather stream: col = jt*640 + k*128 + lw  (tile-major)
        s_lk = (
            i0.reshape(NSC, SC // LT, LT, K)
            .transpose(0, 1, 3, 2)
            .reshape(NSC, NIDX)
        )
        # wrap: element m*16+r of the stream sits at [16c+r, m]; identical
        # for all 8 Q7 cores
        ss = s_lk.reshape(NSC, NIDX // 16, 16)  # (NSC, 320, 16)
        idxg = np.tile(ss.transpose(2, 0, 1), (8, 1, 1))  # (128, NSC, 320)

        # w2[p, j, 2k] = 1, w2[p, j, 2k+1] = frac  (l = j*128 + p)
        # u layout is [t, (2K), o] with blocks 0..K-1 = A_0..A_4 and
        # K..2K-1 = D_0..D_4?  No: ps blocks are [A_k | D_k] interleaved per
        # k, i.e. r=2k is A_k and r=2k+1 is D_k — w2 follows that order.
        w2 = np.empty((128, NT, 2 * K), np.float16)
        frp = fr.reshape(NT, 128, K).transpose(1, 0, 2)  # (128, NT, K)
        w2[:, :, 0::2] = 1.0
        w2[:, :, 1::2] = frp.astype(np.float16)

        in_maps.append(
            {
                "xrow": xrow,
                "idxg": idxg,
                "w2g": w2,
                "wxk": wxk,
                "bias2": bias2,
            }
        )
    return in_maps


def kernel(x, offsets, weight, bias, kernel_size, dilation, stride):
    assert int(kernel_size) == K and int(dilation) == 1 and int(stride) == 1
    if "nc" not in _cache:
        _cache["nc"] = _build_nc()
    nc = _cache["nc"]
    in_maps = _host_prep(x, offsets, weight, bias)
    trace = bool(int(os.environ.get("DC_TRACE", "0")))
    res = bass_utils.run_bass_kernel_spmd(
        nc, in_maps, core_ids=list(range(B)), trace=trace
    )
    _cache["last_exec_time_ns"] = res.exec_time_ns
    out = np.empty((B, O, L_OUT), np.float32)
    for b in range(B):
        out[b] = res.results[b]["out"][:L_OUT, :].astype(np.float32).T
    return out
